# revision 36
# baseline (speedup 1.0000x reference)
"""Memory-Network kernel for 8 Trainium2 NeuronCores (v2).

Data-parallel: batch B=128 split 16-per-core; each core runs its 160
(b, r) sequences end-to-end with no collectives.

Structure vs the v1 baseline (which was LDWEIGHTS-bound at ~96 ns per
matmul, 112 matmuls per LSTM step):
  - The input-side LSTM projection is folded into a host-precomputed
    lookup table xg[v] = W_ih @ emb[v] + bias, shipped as dense fp8
    per-step activations and injected into PSUM with one identity
    matmul per gate m-tile (removes 3 of 7 K-chunks per step).
  - The recurrent matmuls run in fp8 e4m3 DoubleRow mode (K=256 per
    instruction): half the instructions and half the stationary-weight
    bytes per step. Weights and xg are pre-scaled by GSC=512 on host
    (pulls e4m3 out of denormals); the gate activations descale for
    free via the ACT scale operand.
  - Gate activations are batched: m-tiles are packed 3-per-PSUM-bank
    so a step needs 2 sigmoid + 1 tanh(g) + 1 tanh(c) ACT instructions
    instead of 12.
  - The c-state update multiply runs on the otherwise-idle GpSimd
    engine; the rest of the elementwise chain is bf16 on DVE.
  - q/f streams are interleaved on disjoint PSUM bank groups (2+2
    banks ping-pong each) so each stream's matmuls overlap the other's
    activations. The W1 image matmul is spread across the f-tail steps.
"""

import sys

for _p in ("/opt/trn_rl_repo", "/root/.axon_site/_ro/trn_rl_repo"):
    if _p not in sys.path:
        sys.path.insert(0, _p)

import numpy as np
import ml_dtypes

import concourse.bass as bass
import concourse.mybir as mybir
import concourse.tile as tile
from concourse import bacc
from concourse.bass_utils import run_bass_kernel_spmd

BF16 = mybir.dt.bfloat16
F32 = mybir.dt.float32
F8 = mybir.dt.float8e4
DR = mybir.MatmulPerfMode.DoubleRow

NP_BF16 = ml_dtypes.bfloat16
NP_F8 = ml_dtypes.float8_e4m3   # trn2 float8e4: max normal 240

VOCAB, EMB, HID, IMG = 50000, 300, 512, 4096
B, R, LQ, LH = 128, 10, 20, 40
N_CORES = 8
BS = B // N_CORES          # 16 batch items per core
S = BS * R                 # 160 sequences per core
G4 = 4 * HID               # 2048 gate rows
NEG = -1.0e30
GSC = 512.0                # fp8 pre-scale on W_hh and xg (ACT descales)

_STATE = None
_TABLES = None             # host xg lookup tables, set by _prep_shared

# m-tile order (each 128 permuted gate rows): m 0..11 = (j, i/f/o) with
# j-major so chunk0 = j:{0,1}, chunk1 = j:{2,3}; m 12..15 = g_j (tanh).
# PSUM packing: sig chunks 3 m-tiles per bank, tanh chunk 2 per bank.
CHUNKS = [  # (m_list, logical free cols, per-bank slot count, act kind)
    (list(range(0, 6)), 480, 3, "sig"),
    (list(range(6, 12)), 480, 3, "sig"),
    (list(range(12, 16)), 320, 2, "tanh"),
]


def _gate_perm():
    # PyTorch gate row blocks: i:0, f:512, g:1024, o:1536 (chunk j at +128j)
    idx = []
    for j in range(4):
        for base in (0, 512, 1536):            # i_j, f_j, o_j
            idx.append(np.arange(base + 128 * j, base + 128 * j + 128))
    for j in range(4):                          # g_0..g_3
        idx.append(np.arange(1024 + 128 * j, 1024 + 128 * j + 128))
    return np.concatenate(idx)


def _build_program():
    nc = bacc.Bacc(num_swdge_queues=4)

    def din(name, shape, dtype):
        return nc.dram_tensor(name, list(shape), dtype, kind="ExternalInput")

    # per-step gate-permuted xg activations, fp8, pre-scaled by GSC
    xq_d = din("xq", [LQ, 128, 16, S], F8)
    xf_d = din("xf", [LH, 128, 16, S], F8)
    w8q_d = din("w8q", [128, 2, 2, G4], F8)    # [kw, pair, sub, gates]
    w8f_d = din("w8f", [128, 2, 2, G4], F8)
    ident_d = din("ident8", [128, 128], F8)
    w1i_d = din("w1i", [IMG, HID], BF16)
    w1h_d = din("w1h", [HID, HID], BF16)
    b1_d = din("b1", [128, 4], F32)
    w2_d = din("w2", [HID, HID], BF16)
    b2_d = din("b2", [128, 4], F32)
    img_d = din("imgrep", [IMG, S], BF16)
    mask_d = din("mask", [S, S], F32)
    out_d = nc.dram_tensor("out", [HID, S], F32, kind="ExternalOutput")

    with tile.TileContext(nc) as tc:
        with (
            tc.tile_pool(name="consts", bufs=1) as cp,
            tc.tile_pool(name="xgp", bufs=6) as xgp,
            tc.tile_pool(name="hstate", bufs=12) as hp,
            tc.tile_pool(name="cstate", bufs=6) as cpool,
            tc.tile_pool(name="ew", bufs=16) as ew,
            tc.tile_pool(name="w1s", bufs=3) as w1p,
            tc.tile_pool(name="outp", bufs=3) as op,
            tc.tile_pool(name="psq", bufs=2, space="PSUM") as psq,
            tc.tile_pool(name="psf", bufs=2, space="PSUM") as psf,
        ):
            # ---------- constants ----------
            w8 = {}
            for nm, d in (("q", w8q_d), ("f", w8f_d)):
                t_ = cp.tile([128, 2, 2, G4], F8, name=f"w8{nm}", tag=f"w8{nm}")
                nc.sync.dma_start(t_[:], d.ap()[:])
                w8[nm] = t_
            ident = cp.tile([128, 128], F8, name="ident", tag="ident")
            nc.sync.dma_start(ident[:], ident_d.ap()[:])
            w1h_sb = cp.tile([128, 4, HID], BF16, name="w1h_sb", tag="w1h")
            w2_sb = cp.tile([128, 4, HID], BF16, name="w2_sb", tag="w2")
            img_sb = cp.tile([128, IMG // 128, S], BF16, name="img_sb",
                             tag="img")
            b1_sb = cp.tile([128, 4], F32, name="b1_sb", tag="b1")
            nc.sync.dma_start(b1_sb[:], b1_d.ap()[:])
            b2_sb = cp.tile([128, 4], F32, name="b2_sb", tag="b2")
            nc.sync.dma_start(b2_sb[:], b2_d.ap()[:])
            mask_sb = cp.tile([128, 2, S], F32, name="mask_sb", tag="mask")
            nc.sync.dma_start(mask_sb[:, 0, :], mask_d.ap()[0:128, :])
            nc.sync.dma_start(mask_sb[0:S - 128, 1, :], mask_d.ap()[128:S, :])

            def deferred_const_dmas():
                # big tensors not needed until t>=20; keep them off the
                # sync queue during startup so the first xg loads go first
                nc.sync.dma_start(
                    w1h_sb[:],
                    w1h_d.ap().rearrange("(k p) m -> p k m", p=128))
                nc.sync.dma_start(
                    w2_sb[:], w2_d.ap().rearrange("(k p) m -> p k m", p=128))
                nc.sync.dma_start(
                    img_sb[:],
                    img_d.ap().rearrange("(k p) m -> p k m", p=128))

            ident_b = cp.tile([128, 128], BF16, name="identb", tag="identb")
            nc.vector.tensor_copy(ident_b[:], ident[:])

            # ---------- LSTM machinery ----------
            st = {
                "q": dict(T=LQ, xd=xq_d, w=w8["q"], ps=psq, tag="q",
                          h=None, c=None, hb=None),
                "f": dict(T=LH, xd=xf_d, w=w8["f"], ps=psf, tag="f",
                          h=None, c=None, hb=None),
            }

            def lstm_step(s, t):
                tag = s["tag"]
                last = t == s["T"] - 1
                xg = xgp.tile([128, 16, S], F8, name=f"xg{tag}",
                              tag=f"xg{tag}", bufs=3)
                nc.sync.dma_start(xg[:], s["xd"].ap()[t])

                # matmuls: one bank-wide fp8 identity inject per PSUM bank
                # (3 m-tiles at once), then 2 fp8-DR h-matmuls per m-tile
                pg = []
                for mlist, cols, per_bank, _kind in CHUNKS:
                    p = s["ps"].tile([128, 2, cols], F32,
                                     name=f"pg{tag}", tag=f"pg{tag}",
                                     padded_shape=[128, 2, 512])
                    pg.append(p)
                    for b in range(2):
                        m0 = mlist[b * per_bank]
                        nc.tensor.matmul(
                            p[:, b, :], lhsT=ident[:],
                            rhs=xg[:, m0:m0 + per_bank, :],
                            start=True, stop=(t == 0),
                            skip_group_check=True)
                    if t > 0:
                        # pair-major so the p=0 matmuls only depend on the
                        # first half of the previous step's h
                        for pr in range(2):
                            for li, m in enumerate(mlist):
                                o_ap = p[:, li // per_bank,
                                         (li % per_bank) * S:
                                         (li % per_bank + 1) * S]
                                nc.tensor.matmul(
                                    o_ap,
                                    lhsT=s["w"][:, pr, :,
                                                m * 128:(m + 1) * 128],
                                    rhs=s["h"][pr][:],
                                    start=False, stop=(pr == 1),
                                    perf_mode=DR, skip_group_check=True)

                # batched activations (descale by 1/GSC)
                sg = []
                for ci in range(2):
                    sgt = ew.tile([128, 2, 480], BF16, name=f"sg{tag}",
                                  tag=f"sg{tag}", bufs=2)
                    nc.scalar.activation(
                        sgt[:], pg[ci][:, :, 0:480],
                        mybir.ActivationFunctionType.Sigmoid, scale=1.0 / GSC)
                    sg.append(sgt)
                tg = ew.tile([128, 2, 320], BF16, name=f"tg{tag}",
                             tag=f"tg{tag}", bufs=2)
                nc.scalar.activation(
                    tg[:], pg[2][:, :, 0:320],
                    mybir.ActivationFunctionType.Tanh, scale=1.0 / GSC)

                # elementwise chain: c' = f*c + i*tg ; h = o*tanh(c'),
                # finished pair-by-pair so h[:, 0:2] unblocks the next
                # step's p=0 matmuls while pair 1 is still in flight
                c_new = cpool.tile([128, 4, S], F32, name=f"c{tag}",
                                   tag=f"c{tag}", bufs=3)
                tc_ = ew.tile([128, 4, S], BF16, name=f"tc{tag}",
                              tag=f"tc{tag}", bufs=2)
                if last:
                    hb = cp.tile([128, 4, S], BF16, name=f"hb{tag}",
                                 tag=f"hb{tag}")
                    s["hb"] = hb
                    h_pair = [hb[:, 0:2, :], hb[:, 2:4, :]]
                else:
                    h_pair = [hp.tile([128, 2, S], F8, name=f"h{tag}{jp}",
                                      tag=f"h{tag}{jp}", bufs=3)[:]
                              for jp in range(2)]
                for jp in range(2):
                    i_ap = sg[jp][:, :, 0:S]
                    f_ap = sg[jp][:, :, S:2 * S]
                    o_ap = sg[jp][:, :, 2 * S:3 * S]
                    tg_ap = tg[:, jp, :]
                    cpair = c_new[:, 2 * jp:2 * jp + 2, :]
                    if t == 0:
                        nc.vector.tensor_mul(cpair, i_ap, tg_ap)
                    else:
                        m1 = ew.tile([128, 2, S], BF16, name=f"m1{tag}",
                                     tag=f"m1{tag}", bufs=2)
                        nc.vector.tensor_mul(m1[:], i_ap, tg_ap)
                        t2 = ew.tile([128, 2, S], F32, name=f"t2{tag}",
                                     tag=f"t2{tag}", bufs=2)
                        nc.gpsimd.tensor_mul(
                            t2[:], f_ap, s["c"][:, 2 * jp:2 * jp + 2, :])
                        nc.vector.tensor_add(cpair, m1[:], t2[:])
                    tcp = tc_[:, 2 * jp:2 * jp + 2, :]
                    nc.scalar.activation(
                        tcp, cpair, mybir.ActivationFunctionType.Tanh)
                    nc.vector.tensor_mul(h_pair[jp], o_ap, tcp)
                s["h"], s["c"] = h_pair, c_new

            # ---------- W1 (query projection), interleaved with f tail ----
            pw = []

            def w1_alloc():
                for i in range(2):
                    pw.append(psq.tile([128, 2, S], F32, name=f"pw{i}",
                                       tag="pgq", padded_shape=[128, 2, 512]))

            w1c_next = [None]

            def w1_dma(bI):
                w1c = w1p.tile([128, 2, HID], BF16, name="w1c", tag="w1c",
                               bufs=3)
                nc.sync.dma_start(
                    w1c[:],
                    w1i_d.ap()[bI * 256:(bI + 1) * 256, :].rearrange(
                        "(k p) m -> p k m", p=128))
                return w1c

            def w1_img_block(bI, w1c):
                for k8 in range(2):
                    ki = bI * 2 + k8
                    for m in range(4):
                        nc.tensor.matmul(
                            pw[m // 2][:, m % 2, :],
                            lhsT=w1c[:, k8, m * 128:(m + 1) * 128],
                            rhs=img_sb[:, ki, :],
                            start=(ki == 0), stop=False)

            def w1_hq_block(hq):
                for k in range(4):
                    for m in range(4):
                        nc.tensor.matmul(
                            pw[m // 2][:, m % 2, :],
                            lhsT=w1h_sb[:, k, m * 128:(m + 1) * 128],
                            rhs=hq[:, k, :],
                            start=False, stop=(k == 3))

            # ---------- main loop ----------
            for t in range(LH):
                if t < LQ:
                    lstm_step(st["q"], t)
                lstm_step(st["f"], t)
                if t == 1:
                    deferred_const_dmas()
                if t == LQ - 1:
                    w1_alloc()
                    w1c_next[0] = w1_dma(0)
                if LQ <= t < LQ + 16:
                    bI = t - LQ
                    w1c = w1c_next[0]
                    if bI < 15:
                        w1c_next[0] = w1_dma(bI + 1)
                    w1_img_block(bI, w1c)
                if t == LQ + 16:
                    w1_hq_block(st["q"]["hb"])

            # query = tanh(W1 [img; hq] + b1)
            qt_f = []
            qt_b = []
            for m in range(4):
                qf = cp.tile([128, S], F32, name=f"qtf{m}", tag=f"qtf{m}")
                nc.scalar.activation(
                    qf[:], pw[m // 2][:, m % 2, :],
                    mybir.ActivationFunctionType.Tanh, bias=b1_sb[:, m:m + 1])
                qb = cp.tile([128, S], BF16, name=f"qtb{m}", tag=f"qtb{m}")
                nc.vector.tensor_copy(qb[:], qf[:])
                qt_f.append(qf)
                qt_b.append(qb)

            hf = st["f"]["hb"]

            # ---------- attention ----------
            sct = psf.tile([128, 2, S], F32, name="sct", tag="pgf",
                           padded_shape=[128, 2, 512])
            sc0, sc1 = sct[:, 0, :], sct[0:S - 128, 1, :]
            for k in range(4):
                nc.tensor.matmul(sc0, lhsT=qt_b[k][:, 0:128],
                                 rhs=hf[:, k, :], start=(k == 0),
                                 stop=(k == 3))
            for k in range(4):
                nc.tensor.matmul(sc1, lhsT=qt_b[k][:, 128:S],
                                 rhs=hf[:, k, :], start=(k == 0),
                                 stop=(k == 3))

            a_bf = []
            for ti, (scp, npart) in enumerate([(sc0, 128), (sc1, S - 128)]):
                sm = ew.tile([128, S], F32, name="sm", tag="ew")
                nc.vector.tensor_add(sm[:npart], scp, mask_sb[:npart, ti, :])
                nmx = ew.tile([128, 1], F32, name="nmx", tag="red", bufs=4)
                nc.vector.tensor_reduce(
                    nmx[:npart], sm[:npart], mybir.AxisListType.X,
                    mybir.AluOpType.max, negate=True)
                ex = ew.tile([128, S], F32, name="ex", tag="ew")
                nc.scalar.activation(
                    ex[:npart], sm[:npart], mybir.ActivationFunctionType.Exp,
                    bias=nmx[:npart])
                ssum = ew.tile([128, 1], F32, name="ssum", tag="red", bufs=4)
                nc.vector.tensor_reduce(
                    ssum[:npart], ex[:npart], mybir.AxisListType.X,
                    mybir.AluOpType.add)
                rs = ew.tile([128, 1], F32, name="rs", tag="red", bufs=4)
                nc.vector.reciprocal(rs[:npart], ssum[:npart])
                ab = ew.tile([128, S], BF16, name="ab", tag="abf", bufs=8)
                nc.vector.tensor_scalar_mul(ab[:npart], ex[:npart],
                                            rs[:npart])
                a_bf.append(ab)

            # A^T via PE transpose; 2 tiles covering s' 0:128, 128:160
            at = [cp.tile([128, S], BF16, name=f"at{i}", tag=f"at{i}")
                  for i in range(2)]
            blocks = [
                (0, 0, 128, 0, 0),
                (1, 0, 128, 0, 128),
                (0, 128, S, 1, 0),
                (1, 128, S, 1, 128),
            ]
            for (sti, c0, c1, dti, dc) in blocks:
                src = a_bf[sti]
                np_src = 128 if sti == 0 else S - 128
                w = c1 - c0
                pt = psq.tile([128, S], BF16, name="pt", tag="pgq")
                nc.tensor.transpose(
                    pt[0:w, 0:np_src], src[0:np_src, c0:c1],
                    ident_b[0:np_src, 0:np_src])
                nc.vector.tensor_copy(
                    at[dti][0:w, dc:dc + np_src], pt[0:w, 0:np_src])

            # hf token-major [S, 512] as 2 partition tiles
            hft = [cp.tile([128, 4, 128], BF16, name=f"hft{i}", tag=f"hft{i}")
                   for i in range(2)]
            for k in range(4):
                pt = psq.tile([128, S], BF16, name="pt2", tag="pgq")
                nc.tensor.transpose(
                    pt[0:128, 0:128], hf[:, k, 0:128], ident_b[:])
                nc.vector.tensor_copy(hft[0][:, k, :], pt[0:128, 0:128])
                pt = psq.tile([128, S], BF16, name="pt3", tag="pgq")
                nc.tensor.transpose(
                    pt[0:S - 128, 0:128], hf[:, k, 128:S], ident_b[:])
                nc.vector.tensor_copy(
                    hft[1][0:S - 128, k, :], pt[0:S - 128, 0:128])

            # att_hist^T [512, S]: contract over s'
            att_b = []
            pa = [psf.tile([128, 2, S], F32, name=f"pa{i}", tag="pgf",
                           padded_shape=[128, 2, 512]) for i in range(2)]
            for m in range(4):
                pm = pa[m // 2][:, m % 2, :]
                nc.tensor.matmul(pm, lhsT=hft[0][:, m, :], rhs=at[0][:],
                                 start=True, stop=False)
                nc.tensor.matmul(pm, lhsT=hft[1][0:S - 128, m, :],
                                 rhs=at[1][0:S - 128, :],
                                 start=False, stop=True)
                ab2 = ew.tile([128, S], BF16, name="ab2", tag="abf", bufs=8)
                nc.vector.tensor_copy(ab2[:], pm)
                att_b.append(ab2)

            # out = Q + tanh(att @ W2.T + b2), feature-major [512, S]
            po = [psq.tile([128, 2, S], F32, name=f"po{i}", tag="pgq",
                           padded_shape=[128, 2, 512]) for i in range(2)]
            for m in range(4):
                pm = po[m // 2][:, m % 2, :]
                for k in range(4):
                    nc.tensor.matmul(
                        pm, lhsT=w2_sb[:, k, m * 128:(m + 1) * 128],
                        rhs=att_b[k][:], start=(k == 0), stop=(k == 3))
                th = ew.tile([128, S], F32, name="th", tag="ew")
                nc.scalar.activation(
                    th[:], pm, mybir.ActivationFunctionType.Tanh,
                    bias=b2_sb[:, m:m + 1])
                om = op.tile([128, S], F32, name="om", tag="om")
                nc.vector.tensor_add(om[:], th[:], qt_f[m][:])
                nc.sync.dma_start(out_d.ap()[m * 128:(m + 1) * 128, :], om[:])

    nc.compile()
    return nc


def _prep_shared(inp):
    global _TABLES
    f32 = np.float32
    emb = np.asarray(inp["emb"], f32)
    perm = _gate_perm()

    def xg_table(wih, bih, bhh):
        bias = (np.asarray(bih, f32) + np.asarray(bhh, f32))[perm]
        t = emb @ np.asarray(wih, f32)[perm].T * GSC
        t[0, :] = 0.0                      # padding token: emb masked to 0
        t += (bias * GSC)[None, :]
        return np.clip(t, -240.0, 240.0).astype(NP_F8)

    def w8_pack(whh):
        w = np.asarray(whh, f32)[perm].T * GSC     # [512, 2048] permuted
        w = np.clip(w, -240.0, 240.0)
        w = w.reshape(2, 2, 128, G4).transpose(2, 0, 1, 3)
        return np.ascontiguousarray(w).astype(NP_F8)

    _TABLES = {
        "q": xg_table(inp["Wih_q"], inp["bih_q"], inp["bhh_q"]),
        "f": xg_table(inp["Wih_f"], inp["bih_f"], inp["bhh_f"]),
    }

    ident8 = np.zeros((128, 128), NP_F8)
    np.fill_diagonal(ident8, np.float32(1.0))

    W1 = np.asarray(inp["W1"], f32)
    shared = {
        "w8q": w8_pack(inp["Whh_q"]),
        "w8f": w8_pack(inp["Whh_f"]),
        "ident8": ident8,
        "w1i": np.ascontiguousarray(W1[:, :IMG].T).astype(NP_BF16),
        "w1h": np.ascontiguousarray(W1[:, IMG:].T).astype(NP_BF16),
        "b1": np.ascontiguousarray(
            np.asarray(inp["b1"], f32).reshape(4, 128).T),
        "w2": np.ascontiguousarray(np.asarray(inp["W2"], f32).T).astype(
            NP_BF16),
        "b2": np.ascontiguousarray(
            np.asarray(inp["b2"], f32).reshape(4, 128).T),
    }
    n = np.arange(S)
    mask = np.where(
        (n[:, None] // R == n[None, :] // R)
        & (n[None, :] % R <= n[:, None] % R),
        np.float32(0.0), np.float32(NEG))
    shared["mask"] = np.ascontiguousarray(mask.astype(f32))
    return shared


def _prep_core(inp, core):
    sl = slice(core * BS, (core + 1) * BS)

    def xg_stream(tokens, table, L):
        toks = np.asarray(tokens[sl], np.int64).reshape(S, L)   # [160, L]
        g = table[toks]                       # [160, L, 2048] fp8
        g = g.transpose(1, 2, 0)              # [L, 2048, 160]
        g = g.reshape(L, 16, 128, S).transpose(0, 2, 1, 3)
        return np.ascontiguousarray(g)        # [L, 128, 16, 160]

    img = np.asarray(inp["img_features"], np.float32)[sl]       # [16, 4096]
    img_rep = np.repeat(img, R, axis=0).T                       # [4096, 160]
    return {
        "xq": xg_stream(inp["questions"], _TABLES["q"], LQ),
        "xf": xg_stream(inp["history"], _TABLES["f"], LH),
        "imgrep": np.ascontiguousarray(img_rep).astype(NP_BF16),
    }


def kernel(**inputs) -> np.ndarray:
    global _STATE
    if _STATE is None:
        _STATE = _build_program()
    nc = _STATE

    shared = _prep_shared(inputs)
    in_maps = []
    for c in range(N_CORES):
        m = dict(shared)
        m.update(_prep_core(inputs, c))
        in_maps.append(m)

    res = run_bass_kernel_spmd(nc, in_maps, core_ids=list(range(N_CORES)))
    outs = []
    for c in range(N_CORES):
        o = np.asarray(res.results[c]["out"], np.float32)   # [512, 160]
        outs.append(o.T.reshape(BS, R, HID))
    return np.concatenate(outs, axis=0)                      # [128, 10, 512]


# revision 39
# speedup vs baseline: 1.0229x; 1.0229x over previous
"""Memory-Network kernel for 8 Trainium2 NeuronCores (v2).

Data-parallel: batch B=128 split 16-per-core; each core runs its 160
(b, r) sequences end-to-end with no collectives.

Structure vs the v1 baseline (which was LDWEIGHTS-bound at ~96 ns per
matmul, 112 matmuls per LSTM step):
  - The input-side LSTM projection is folded into a host-precomputed
    lookup table xg[v] = W_ih @ emb[v] + bias, shipped as dense fp8
    per-step activations and injected into PSUM with one identity
    matmul per gate m-tile (removes 3 of 7 K-chunks per step).
  - The recurrent matmuls run in fp8 e4m3 DoubleRow mode (K=256 per
    instruction): half the instructions and half the stationary-weight
    bytes per step. Weights and xg are pre-scaled by GSC=512 on host
    (pulls e4m3 out of denormals); the gate activations descale for
    free via the ACT scale operand.
  - Gate activations are batched: m-tiles are packed 3-per-PSUM-bank
    so a step needs 2 sigmoid + 1 tanh(g) + 1 tanh(c) ACT instructions
    instead of 12.
  - The c-state update multiply runs on the otherwise-idle GpSimd
    engine; the rest of the elementwise chain is bf16 on DVE.
  - q/f streams are interleaved on disjoint PSUM bank groups (2+2
    banks ping-pong each) so each stream's matmuls overlap the other's
    activations. The W1 image matmul is spread across the f-tail steps.
"""

import sys

for _p in ("/opt/trn_rl_repo", "/root/.axon_site/_ro/trn_rl_repo"):
    if _p not in sys.path:
        sys.path.insert(0, _p)

import numpy as np
import ml_dtypes

import concourse.bass as bass
import concourse.mybir as mybir
import concourse.tile as tile
from concourse import bacc
from concourse.bass_utils import run_bass_kernel_spmd

BF16 = mybir.dt.bfloat16
F32 = mybir.dt.float32
F8 = mybir.dt.float8e4
DR = mybir.MatmulPerfMode.DoubleRow

NP_BF16 = ml_dtypes.bfloat16
NP_F8 = ml_dtypes.float8_e4m3   # trn2 float8e4: max normal 240

VOCAB, EMB, HID, IMG = 50000, 300, 512, 4096
B, R, LQ, LH = 128, 10, 20, 40
N_CORES = 8
BS = B // N_CORES          # 16 batch items per core
S = BS * R                 # 160 sequences per core
G4 = 4 * HID               # 2048 gate rows
NEG = -1.0e30
GSC = 512.0                # fp8 pre-scale on W_hh and xg (ACT descales)

_STATE = None
_TABLES = None             # host xg lookup tables, set by _prep_shared

# m-tile order (each 128 permuted gate rows): m 0..11 = (j, i/f/o) with
# j-major so chunk0 = j:{0,1}, chunk1 = j:{2,3}; m 12..15 = g_j (tanh).
# PSUM packing: sig chunks 3 m-tiles per bank, tanh chunk 2 per bank.
CHUNKS = [  # (m_list, logical free cols, per-bank slot count, act kind)
    (list(range(0, 6)), 480, 3, "sig"),
    (list(range(6, 12)), 480, 3, "sig"),
    (list(range(12, 16)), 320, 2, "tanh"),
]


def _gate_perm():
    # PyTorch gate row blocks: i:0, f:512, g:1024, o:1536 (chunk j at +128j)
    idx = []
    for j in range(4):
        for base in (0, 512, 1536):            # i_j, f_j, o_j
            idx.append(np.arange(base + 128 * j, base + 128 * j + 128))
    for j in range(4):                          # g_0..g_3
        idx.append(np.arange(1024 + 128 * j, 1024 + 128 * j + 128))
    return np.concatenate(idx)


def _build_program():
    nc = bacc.Bacc(num_swdge_queues=4)

    def din(name, shape, dtype):
        return nc.dram_tensor(name, list(shape), dtype, kind="ExternalInput")

    # per-step gate-permuted xg activations, fp8, pre-scaled by GSC
    xq_d = din("xq", [LQ, 128, 16, S], F8)
    xf_d = din("xf", [LH, 128, 16, S], F8)
    w8q_d = din("w8q", [128, 2, 2, G4], F8)    # [kw, pair, sub, gates]
    w8f_d = din("w8f", [128, 2, 2, G4], F8)
    ident_d = din("ident8", [128, 128], F8)
    w1i_d = din("w1i", [IMG, HID], BF16)
    w1h_d = din("w1h", [HID, HID], BF16)
    b1_d = din("b1", [128, 4], F32)
    w2_d = din("w2", [HID, HID], BF16)
    b2_d = din("b2", [128, 4], F32)
    img_d = din("imgrep", [IMG, S], BF16)
    mask_d = din("mask", [S, S], F32)
    out_d = nc.dram_tensor("out", [HID, S], F32, kind="ExternalOutput")

    with tile.TileContext(nc) as tc:
        with (
            tc.tile_pool(name="consts", bufs=1) as cp,
            tc.tile_pool(name="xgp", bufs=6) as xgp,
            tc.tile_pool(name="hstate", bufs=12) as hp,
            tc.tile_pool(name="cstate", bufs=6) as cpool,
            tc.tile_pool(name="ew", bufs=16) as ew,
            tc.tile_pool(name="w1s", bufs=3) as w1p,
            tc.tile_pool(name="outp", bufs=3) as op,
            tc.tile_pool(name="psq", bufs=2, space="PSUM") as psq,
            tc.tile_pool(name="psf", bufs=2, space="PSUM") as psf,
        ):
            # ---------- constants ----------
            w8 = {}
            for nm, d in (("q", w8q_d), ("f", w8f_d)):
                t_ = cp.tile([128, 2, 2, G4], F8, name=f"w8{nm}", tag=f"w8{nm}")
                nc.sync.dma_start(t_[:], d.ap()[:])
                w8[nm] = t_
            ident = cp.tile([128, 128], F8, name="ident", tag="ident")
            nc.sync.dma_start(ident[:], ident_d.ap()[:])
            w1h_sb = cp.tile([128, 4, HID], BF16, name="w1h_sb", tag="w1h")
            w2_sb = cp.tile([128, 4, HID], BF16, name="w2_sb", tag="w2")
            img_sb = cp.tile([128, IMG // 128, S], BF16, name="img_sb",
                             tag="img")
            b1_sb = cp.tile([128, 4], F32, name="b1_sb", tag="b1")
            nc.sync.dma_start(b1_sb[:], b1_d.ap()[:])
            b2_sb = cp.tile([128, 4], F32, name="b2_sb", tag="b2")
            nc.sync.dma_start(b2_sb[:], b2_d.ap()[:])
            mask_sb = cp.tile([128, 2, S], F32, name="mask_sb", tag="mask")
            nc.sync.dma_start(mask_sb[:, 0, :], mask_d.ap()[0:128, :])
            nc.sync.dma_start(mask_sb[0:S - 128, 1, :], mask_d.ap()[128:S, :])

            def deferred_const_dmas():
                # big tensors not needed until t>=20; keep them off the
                # sync queue during startup so the first xg loads go first
                nc.sync.dma_start(
                    w1h_sb[:],
                    w1h_d.ap().rearrange("(k p) m -> p k m", p=128))
                nc.sync.dma_start(
                    w2_sb[:], w2_d.ap().rearrange("(k p) m -> p k m", p=128))
                nc.sync.dma_start(
                    img_sb[:],
                    img_d.ap().rearrange("(k p) m -> p k m", p=128))

            ident_b = cp.tile([128, 128], BF16, name="identb", tag="identb")
            nc.vector.tensor_copy(ident_b[:], ident[:])

            # ---------- LSTM machinery ----------
            st = {
                "q": dict(T=LQ, xd=xq_d, w=w8["q"], ps=psq, tag="q",
                          h=None, c=None, hb=None),
                "f": dict(T=LH, xd=xf_d, w=w8["f"], ps=psf, tag="f",
                          h=None, c=None, hb=None),
            }

            def lstm_step(s, t):
                tag = s["tag"]
                last = t == s["T"] - 1
                xg = xgp.tile([128, 16, S], F8, name=f"xg{tag}",
                              tag=f"xg{tag}", bufs=3)
                nc.sync.dma_start(xg[:], s["xd"].ap()[t])

                # matmuls: one bank-wide fp8 identity inject per PSUM bank
                # (3 m-tiles at once), then 2 fp8-DR h-matmuls per m-tile.
                # The tanh-g chunk is emitted FIRST so its activation (on
                # the next step's critical path via i*tanh(g)) completes
                # while the sigmoid chunks' matmuls still stream.
                pg = [None, None, None]
                for ci in (2, 0, 1):
                    mlist, cols, per_bank, _kind = CHUNKS[ci]
                    p = s["ps"].tile([128, 2, cols], F32,
                                     name=f"pg{tag}", tag=f"pg{tag}",
                                     padded_shape=[128, 2, 512])
                    pg[ci] = p
                    for b in range(2):
                        m0 = mlist[b * per_bank]
                        nc.tensor.matmul(
                            p[:, b, :], lhsT=ident[:],
                            rhs=xg[:, m0:m0 + per_bank, :],
                            start=True, stop=(t == 0),
                            skip_group_check=True)
                    if t > 0:
                        # pair-major so the p=0 matmuls only depend on the
                        # first half of the previous step's h
                        for pr in range(2):
                            for li, m in enumerate(mlist):
                                o_ap = p[:, li // per_bank,
                                         (li % per_bank) * S:
                                         (li % per_bank + 1) * S]
                                nc.tensor.matmul(
                                    o_ap,
                                    lhsT=s["w"][:, pr, :,
                                                m * 128:(m + 1) * 128],
                                    rhs=s["h"][:, 2 * pr:2 * pr + 2, :],
                                    start=False, stop=(pr == 1),
                                    perf_mode=DR, skip_group_check=True)

                # batched activations (descale by 1/GSC); tanh-g first to
                # match the matmul emission order
                tg = ew.tile([128, 2, 320], BF16, name=f"tg{tag}",
                             tag=f"tg{tag}", bufs=2)
                nc.scalar.activation(
                    tg[:], pg[2][:, :, 0:320],
                    mybir.ActivationFunctionType.Tanh, scale=1.0 / GSC)
                sg = []
                for ci in range(2):
                    sgt = ew.tile([128, 2, 480], BF16, name=f"sg{tag}",
                                  tag=f"sg{tag}", bufs=2)
                    nc.scalar.activation(
                        sgt[:], pg[ci][:, :, 0:480],
                        mybir.ActivationFunctionType.Sigmoid, scale=1.0 / GSC)
                    sg.append(sgt)

                # elementwise chain: c' = f*c + i*tg ; h = o*tanh(c'),
                # finished pair-by-pair so h[:, 0:2] unblocks the next
                # step's p=0 matmuls while pair 1 is still in flight
                c_new = cpool.tile([128, 4, S], F32, name=f"c{tag}",
                                   tag=f"c{tag}", bufs=3)
                tc_ = ew.tile([128, 4, S], BF16, name=f"tc{tag}",
                              tag=f"tc{tag}", bufs=2)
                if last:
                    h_new = cp.tile([128, 4, S], BF16, name=f"hb{tag}",
                                    tag=f"hb{tag}")
                    s["hb"] = h_new
                else:
                    h_new = hp.tile([128, 4, S], F8, name=f"h{tag}",
                                    tag=f"h{tag}", bufs=3)
                for jp in range(2):
                    i_ap = sg[jp][:, :, 0:S]
                    f_ap = sg[jp][:, :, S:2 * S]
                    o_ap = sg[jp][:, :, 2 * S:3 * S]
                    tg_ap = tg[:, jp, :]
                    cpair = c_new[:, 2 * jp:2 * jp + 2, :]
                    if t == 0:
                        nc.vector.tensor_mul(cpair, i_ap, tg_ap)
                    else:
                        m1 = ew.tile([128, 2, S], BF16, name=f"m1{tag}",
                                     tag=f"m1{tag}", bufs=2)
                        nc.vector.tensor_mul(m1[:], i_ap, tg_ap)
                        t2 = ew.tile([128, 2, S], F32, name=f"t2{tag}",
                                     tag=f"t2{tag}", bufs=2)
                        nc.gpsimd.tensor_mul(
                            t2[:], f_ap, s["c"][:, 2 * jp:2 * jp + 2, :])
                        nc.vector.tensor_add(cpair, m1[:], t2[:])
                    tcp = tc_[:, 2 * jp:2 * jp + 2, :]
                    nc.scalar.activation(
                        tcp, cpair, mybir.ActivationFunctionType.Tanh)
                    nc.vector.tensor_mul(
                        h_new[:, 2 * jp:2 * jp + 2, :], o_ap, tcp)
                s["h"], s["c"] = h_new, c_new

            # ---------- W1 (query projection), interleaved with f tail ----
            pw = []

            def w1_alloc():
                for i in range(2):
                    pw.append(psq.tile([128, 2, S], F32, name=f"pw{i}",
                                       tag="pgq", padded_shape=[128, 2, 512]))

            w1c_next = [None]

            def w1_dma(bI):
                w1c = w1p.tile([128, 2, HID], BF16, name="w1c", tag="w1c",
                               bufs=3)
                nc.sync.dma_start(
                    w1c[:],
                    w1i_d.ap()[bI * 256:(bI + 1) * 256, :].rearrange(
                        "(k p) m -> p k m", p=128))
                return w1c

            def w1_img_block(bI, w1c):
                for k8 in range(2):
                    ki = bI * 2 + k8
                    for m in range(4):
                        nc.tensor.matmul(
                            pw[m // 2][:, m % 2, :],
                            lhsT=w1c[:, k8, m * 128:(m + 1) * 128],
                            rhs=img_sb[:, ki, :],
                            start=(ki == 0), stop=False)

            def w1_hq_block(hq):
                for k in range(4):
                    for m in range(4):
                        nc.tensor.matmul(
                            pw[m // 2][:, m % 2, :],
                            lhsT=w1h_sb[:, k, m * 128:(m + 1) * 128],
                            rhs=hq[:, k, :],
                            start=False, stop=(k == 3))

            # ---------- main loop ----------
            for t in range(LH):
                if t < LQ:
                    lstm_step(st["q"], t)
                lstm_step(st["f"], t)
                if t == 1:
                    deferred_const_dmas()
                if t == LQ - 1:
                    w1_alloc()
                    w1c_next[0] = w1_dma(0)
                if LQ <= t < LQ + 16:
                    bI = t - LQ
                    w1c = w1c_next[0]
                    if bI < 15:
                        w1c_next[0] = w1_dma(bI + 1)
                    w1_img_block(bI, w1c)
                if t == LQ + 16:
                    w1_hq_block(st["q"]["hb"])

            # query = tanh(W1 [img; hq] + b1)
            qt_f = []
            qt_b = []
            for m in range(4):
                qf = cp.tile([128, S], F32, name=f"qtf{m}", tag=f"qtf{m}")
                nc.scalar.activation(
                    qf[:], pw[m // 2][:, m % 2, :],
                    mybir.ActivationFunctionType.Tanh, bias=b1_sb[:, m:m + 1])
                qb = cp.tile([128, S], BF16, name=f"qtb{m}", tag=f"qtb{m}")
                nc.vector.tensor_copy(qb[:], qf[:])
                qt_f.append(qf)
                qt_b.append(qb)

            hf = st["f"]["hb"]

            # ---------- attention ----------
            sct = psf.tile([128, 2, S], F32, name="sct", tag="pgf",
                           padded_shape=[128, 2, 512])
            sc0, sc1 = sct[:, 0, :], sct[0:S - 128, 1, :]
            for k in range(4):
                nc.tensor.matmul(sc0, lhsT=qt_b[k][:, 0:128],
                                 rhs=hf[:, k, :], start=(k == 0),
                                 stop=(k == 3))
            for k in range(4):
                nc.tensor.matmul(sc1, lhsT=qt_b[k][:, 128:S],
                                 rhs=hf[:, k, :], start=(k == 0),
                                 stop=(k == 3))

            a_bf = []
            for ti, (scp, npart) in enumerate([(sc0, 128), (sc1, S - 128)]):
                sm = ew.tile([128, S], F32, name="sm", tag="ew")
                nc.vector.tensor_add(sm[:npart], scp, mask_sb[:npart, ti, :])
                nmx = ew.tile([128, 1], F32, name="nmx", tag="red", bufs=4)
                nc.vector.tensor_reduce(
                    nmx[:npart], sm[:npart], mybir.AxisListType.X,
                    mybir.AluOpType.max, negate=True)
                ex = ew.tile([128, S], F32, name="ex", tag="ew")
                nc.scalar.activation(
                    ex[:npart], sm[:npart], mybir.ActivationFunctionType.Exp,
                    bias=nmx[:npart])
                ssum = ew.tile([128, 1], F32, name="ssum", tag="red", bufs=4)
                nc.vector.tensor_reduce(
                    ssum[:npart], ex[:npart], mybir.AxisListType.X,
                    mybir.AluOpType.add)
                rs = ew.tile([128, 1], F32, name="rs", tag="red", bufs=4)
                nc.vector.reciprocal(rs[:npart], ssum[:npart])
                ab = ew.tile([128, S], BF16, name="ab", tag="abf", bufs=8)
                nc.vector.tensor_scalar_mul(ab[:npart], ex[:npart],
                                            rs[:npart])
                a_bf.append(ab)

            # A^T via PE transpose; 2 tiles covering s' 0:128, 128:160
            at = [cp.tile([128, S], BF16, name=f"at{i}", tag=f"at{i}")
                  for i in range(2)]
            blocks = [
                (0, 0, 128, 0, 0),
                (1, 0, 128, 0, 128),
                (0, 128, S, 1, 0),
                (1, 128, S, 1, 128),
            ]
            for (sti, c0, c1, dti, dc) in blocks:
                src = a_bf[sti]
                np_src = 128 if sti == 0 else S - 128
                w = c1 - c0
                pt = psq.tile([128, S], BF16, name="pt", tag="pgq")
                nc.tensor.transpose(
                    pt[0:w, 0:np_src], src[0:np_src, c0:c1],
                    ident_b[0:np_src, 0:np_src])
                nc.vector.tensor_copy(
                    at[dti][0:w, dc:dc + np_src], pt[0:w, 0:np_src])

            # hf token-major [S, 512] as 2 partition tiles
            hft = [cp.tile([128, 4, 128], BF16, name=f"hft{i}", tag=f"hft{i}")
                   for i in range(2)]
            for k in range(4):
                pt = psq.tile([128, S], BF16, name="pt2", tag="pgq")
                nc.tensor.transpose(
                    pt[0:128, 0:128], hf[:, k, 0:128], ident_b[:])
                nc.vector.tensor_copy(hft[0][:, k, :], pt[0:128, 0:128])
                pt = psq.tile([128, S], BF16, name="pt3", tag="pgq")
                nc.tensor.transpose(
                    pt[0:S - 128, 0:128], hf[:, k, 128:S], ident_b[:])
                nc.vector.tensor_copy(
                    hft[1][0:S - 128, k, :], pt[0:S - 128, 0:128])

            # att_hist^T [512, S]: contract over s'
            att_b = []
            pa = [psf.tile([128, 2, S], F32, name=f"pa{i}", tag="pgf",
                           padded_shape=[128, 2, 512]) for i in range(2)]
            for m in range(4):
                pm = pa[m // 2][:, m % 2, :]
                nc.tensor.matmul(pm, lhsT=hft[0][:, m, :], rhs=at[0][:],
                                 start=True, stop=False)
                nc.tensor.matmul(pm, lhsT=hft[1][0:S - 128, m, :],
                                 rhs=at[1][0:S - 128, :],
                                 start=False, stop=True)
                ab2 = ew.tile([128, S], BF16, name="ab2", tag="abf", bufs=8)
                nc.vector.tensor_copy(ab2[:], pm)
                att_b.append(ab2)

            # out = Q + tanh(att @ W2.T + b2), feature-major [512, S]
            po = [psq.tile([128, 2, S], F32, name=f"po{i}", tag="pgq",
                           padded_shape=[128, 2, 512]) for i in range(2)]
            for m in range(4):
                pm = po[m // 2][:, m % 2, :]
                for k in range(4):
                    nc.tensor.matmul(
                        pm, lhsT=w2_sb[:, k, m * 128:(m + 1) * 128],
                        rhs=att_b[k][:], start=(k == 0), stop=(k == 3))
                th = ew.tile([128, S], F32, name="th", tag="ew")
                nc.scalar.activation(
                    th[:], pm, mybir.ActivationFunctionType.Tanh,
                    bias=b2_sb[:, m:m + 1])
                om = op.tile([128, S], F32, name="om", tag="om")
                nc.vector.tensor_add(om[:], th[:], qt_f[m][:])
                nc.sync.dma_start(out_d.ap()[m * 128:(m + 1) * 128, :], om[:])

    nc.compile()
    return nc


def _prep_shared(inp):
    global _TABLES
    f32 = np.float32
    emb = np.asarray(inp["emb"], f32)
    perm = _gate_perm()

    def xg_table(wih, bih, bhh):
        bias = (np.asarray(bih, f32) + np.asarray(bhh, f32))[perm]
        t = emb @ np.asarray(wih, f32)[perm].T * GSC
        t[0, :] = 0.0                      # padding token: emb masked to 0
        t += (bias * GSC)[None, :]
        return np.clip(t, -240.0, 240.0).astype(NP_F8)

    def w8_pack(whh):
        w = np.asarray(whh, f32)[perm].T * GSC     # [512, 2048] permuted
        w = np.clip(w, -240.0, 240.0)
        w = w.reshape(2, 2, 128, G4).transpose(2, 0, 1, 3)
        return np.ascontiguousarray(w).astype(NP_F8)

    _TABLES = {
        "q": xg_table(inp["Wih_q"], inp["bih_q"], inp["bhh_q"]),
        "f": xg_table(inp["Wih_f"], inp["bih_f"], inp["bhh_f"]),
    }

    ident8 = np.zeros((128, 128), NP_F8)
    np.fill_diagonal(ident8, np.float32(1.0))

    W1 = np.asarray(inp["W1"], f32)
    shared = {
        "w8q": w8_pack(inp["Whh_q"]),
        "w8f": w8_pack(inp["Whh_f"]),
        "ident8": ident8,
        "w1i": np.ascontiguousarray(W1[:, :IMG].T).astype(NP_BF16),
        "w1h": np.ascontiguousarray(W1[:, IMG:].T).astype(NP_BF16),
        "b1": np.ascontiguousarray(
            np.asarray(inp["b1"], f32).reshape(4, 128).T),
        "w2": np.ascontiguousarray(np.asarray(inp["W2"], f32).T).astype(
            NP_BF16),
        "b2": np.ascontiguousarray(
            np.asarray(inp["b2"], f32).reshape(4, 128).T),
    }
    n = np.arange(S)
    mask = np.where(
        (n[:, None] // R == n[None, :] // R)
        & (n[None, :] % R <= n[:, None] % R),
        np.float32(0.0), np.float32(NEG))
    shared["mask"] = np.ascontiguousarray(mask.astype(f32))
    return shared


def _prep_core(inp, core):
    sl = slice(core * BS, (core + 1) * BS)

    def xg_stream(tokens, table, L):
        toks = np.asarray(tokens[sl], np.int64).reshape(S, L)   # [160, L]
        g = table[toks]                       # [160, L, 2048] fp8
        g = g.transpose(1, 2, 0)              # [L, 2048, 160]
        g = g.reshape(L, 16, 128, S).transpose(0, 2, 1, 3)
        return np.ascontiguousarray(g)        # [L, 128, 16, 160]

    img = np.asarray(inp["img_features"], np.float32)[sl]       # [16, 4096]
    img_rep = np.repeat(img, R, axis=0).T                       # [4096, 160]
    return {
        "xq": xg_stream(inp["questions"], _TABLES["q"], LQ),
        "xf": xg_stream(inp["history"], _TABLES["f"], LH),
        "imgrep": np.ascontiguousarray(img_rep).astype(NP_BF16),
    }


def kernel(**inputs) -> np.ndarray:
    global _STATE
    if _STATE is None:
        _STATE = _build_program()
    nc = _STATE

    shared = _prep_shared(inputs)
    in_maps = []
    for c in range(N_CORES):
        m = dict(shared)
        m.update(_prep_core(inputs, c))
        in_maps.append(m)

    res = run_bass_kernel_spmd(nc, in_maps, core_ids=list(range(N_CORES)))
    outs = []
    for c in range(N_CORES):
        o = np.asarray(res.results[c]["out"], np.float32)   # [512, 160]
        outs.append(o.T.reshape(BS, R, HID))
    return np.concatenate(outs, axis=0)                      # [128, 10, 512]
